# revision 1
# baseline (speedup 1.0000x reference)
"""Trainium2 Bass kernel for HandmadeConv2d.

Conv2d NCHW, valid padding, stride 1, no bias:
  x: (32, 128, 64, 64) f32, weights: (256, 128, 3, 3) f32 -> out: (32, 256, 62, 62) f32

Sharding: data-parallel over batch, 4 images per core across 8 NeuronCores;
weights replicated.

Per core the conv is computed as 9 accumulating matmuls per output tile:
  out[oc, (oh,ow)] += W[kh,kw][ic, oc].T @ x[ic, (oh+kh, ow+kw)]
with ic=128 as the PE contraction dim, oc split into 2 chunks of 128
(PSUM partition dim), and spatial tiled as 8 output rows x 62 cols = 496
moving-operand elements (<=512 fp32 limit, fits one PSUM bank).

Weight [ic, oc] tiles are built on-chip with PE transposes of the
contiguously-DMA'd [oc, ic] layout.

Matmul dtype modes (BASS_CONV_MODE env var):
  fp32      - native fp32 matmul (4 cycles/row), ~1e-7 rel err
  fp32r     - single-pass rounded fp32 (1 cycle/row), ~1.5e-4 rel err
  fp32rsplit- hi/lo fp32r decomposition, 3 matmuls (~3 cycles/row), ~1e-7 rel err
  bf16split - hi/lo bf16 decomposition, 3 matmuls (~3 cycles/row), ~5e-6 rel err
"""

import os
import warnings

warnings.filterwarnings("ignore")

import numpy as np

N_CORES = 8
NIMG = 4  # images per core
IC = 128
OC = 256
H = W = 64
OH = OW = 62
P = 128

MODE = os.environ.get("BASS_CONV_MODE", "fp32rsplit")

_NC_CACHE = {}


def _row_groups():
    groups = []
    r = 0
    while r < OH:
        nr = min(8, OH - r)
        groups.append((r, nr))
        r += nr
    return groups


def build_nc(mode):
    import concourse.bacc as bacc
    import concourse.mybir as mybir
    import concourse.tile as tile
    from concourse.masks import make_identity

    f32 = mybir.dt.float32
    f32r = mybir.dt.float32r
    bf16 = mybir.dt.bfloat16

    nc = bacc.Bacc("TRN2", target_bir_lowering=False, debug=False)
    x = nc.dram_tensor("x", [NIMG, IC, H, W], f32, kind="ExternalInput")
    w = nc.dram_tensor("w", [OC, IC, 3, 3], f32, kind="ExternalInput")
    out = nc.dram_tensor("out", [NIMG, OC, OH, OW], f32, kind="ExternalOutput")

    split = mode in ("fp32rsplit", "bf16split")
    if mode == "fp32":
        mm_dt = f32
    elif mode == "fp32r":
        mm_dt = f32r
    elif mode == "fp32rsplit":
        mm_dt = f32r
    elif mode == "bf16split":
        mm_dt = bf16
    else:
        raise ValueError(mode)

    groups = _row_groups()

    with tile.TileContext(nc) as tc:
        with (
            tc.tile_pool(name="wstage", bufs=1) as wstage,
            tc.tile_pool(name="wtiles", bufs=1) as wtiles,
            tc.tile_pool(name="xstage", bufs=2) as xstage,
            tc.tile_pool(name="xconv", bufs=2) as xconv,
            tc.tile_pool(name="osb", bufs=8) as osb,
            tc.tile_pool(name="consts", bufs=1) as consts,
            tc.tile_pool(name="pst", bufs=2, space="PSUM") as pst,
            tc.tile_pool(name="psmm", bufs=6, space="PSUM") as psmm,
        ):
            # ---- weights: DMA contiguous [oc, ic*9], PE-transpose to [ic, oc] tiles
            ident = consts.tile([P, P], f32)
            make_identity(nc, ident[:])

            wsb = wstage.tile([P, 2, IC * 9], f32)
            for c in range(2):
                nc.sync.dma_start(
                    wsb[:, c, :],
                    w[:][c * P : (c + 1) * P].rearrange("o i h k -> o (i h k)"),
                )
            wsb_r = wsb[:].rearrange("o c (i k) -> o c i k", k=9)

            wt_hi = wtiles.tile([P, 2, 9, P], mm_dt, tag="wt_hi")
            if split:
                wt_lo = wtiles.tile([P, 2, 9, P], mm_dt, tag="wt_lo")
                wtmp = wtiles.tile([P, P], f32, tag="wtmp", bufs=2)

            for c in range(2):
                for k in range(9):
                    ptile = pst.tile([P, P], f32)
                    nc.tensor.transpose(ptile[:], wsb_r[:, c, :, k], ident[:])
                    hi_slice = wt_hi[:, c, k, :]
                    nc.vector.tensor_copy(hi_slice, ptile[:])
                    if split:
                        if mode == "fp32rsplit":
                            hi32 = hi_slice.bitcast(f32)
                        else:
                            hi32 = wtiles.tile([P, P], f32, tag="hi32", bufs=2)[:]
                            nc.vector.tensor_copy(hi32, hi_slice)
                        lo32 = wtmp[:]
                        nc.vector.tensor_sub(lo32, ptile[:], hi32)
                        nc.vector.tensor_copy(wt_lo[:, c, k, :], lo32)
                        wtmp = wtiles.tile([P, P], f32, tag="wtmp", bufs=2)

            # ---- main loop over images
            for n in range(NIMG):
                stage = xstage.tile([P, H, W], f32, tag="xs")
                nc.sync.dma_start(stage[:], x[:][n])

                if mode == "fp32":
                    xterms = [stage]
                elif mode == "fp32r":
                    xhi = xconv.tile([P, H, W], f32r, tag="xhi")
                    nc.vector.tensor_copy(xhi[:], stage[:])
                    xterms = [xhi]
                else:
                    xhi = xconv.tile([P, H, W], mm_dt, tag="xhi")
                    nc.vector.tensor_copy(xhi[:], stage[:])
                    if mode == "fp32rsplit":
                        hi32v = xhi[:].bitcast(f32)
                    else:
                        hi32 = xconv.tile([P, H, W], f32, tag="xhi32")
                        nc.vector.tensor_copy(hi32[:], xhi[:])
                        hi32v = hi32[:]
                    lo32 = xconv.tile([P, H, W], f32, tag="xlo32")
                    nc.vector.tensor_sub(lo32[:], stage[:], hi32v)
                    xlo = xconv.tile([P, H, W], mm_dt, tag="xlo")
                    nc.vector.tensor_copy(xlo[:], lo32[:])
                    xterms = [xhi, xlo]

                # (weight_tile, x_tile) products to accumulate
                if split:
                    terms = [(wt_hi, xterms[0]), (wt_hi, xterms[1]), (wt_lo, xterms[0])]
                else:
                    terms = [(wt_hi, xterms[0])]

                for c in range(2):
                    for r0, nr in groups:
                        ps_t = psmm.tile([P, 8 * OW], f32, tag="mm")
                        nmm = len(terms) * 9
                        i = 0
                        for wt, xt in terms:
                            for kh in range(3):
                                for kw in range(3):
                                    nc.tensor.matmul(
                                        ps_t[:, : nr * OW],
                                        wt[:, c, kh * 3 + kw, :],
                                        xt[:, r0 + kh : r0 + kh + nr, kw : kw + OW],
                                        start=(i == 0),
                                        stop=(i == nmm - 1),
                                    )
                                    i += 1
                        ob = osb.tile([P, 8 * OW], f32, tag="ob")
                        nc.any.tensor_copy(ob[:, : nr * OW], ps_t[:, : nr * OW])
                        nc.sync.dma_start(
                            out[:][n, c * P : (c + 1) * P, r0 : r0 + nr, :],
                            ob[:, : nr * OW].rearrange("p (r q) -> p r q", q=OW),
                        )

    nc.compile()
    return nc


def get_nc(mode=None):
    mode = mode or MODE
    if mode not in _NC_CACHE:
        _NC_CACHE[mode] = build_nc(mode)
    return _NC_CACHE[mode]


def kernel(x, weights, _trace=False, _mode=None):
    from concourse.bass_utils import run_bass_kernel_spmd

    nc = get_nc(_mode)
    x = np.ascontiguousarray(np.asarray(x), dtype=np.float32)
    weights = np.ascontiguousarray(np.asarray(weights), dtype=np.float32)
    in_maps = [
        {"x": x[i * NIMG : (i + 1) * NIMG], "w": weights} for i in range(N_CORES)
    ]
    res = run_bass_kernel_spmd(
        nc, in_maps, core_ids=list(range(N_CORES)), trace=_trace
    )
    out = np.concatenate([r["out"] for r in res.results], axis=0)
    if _trace:
        kernel.last_results = res
    return out


kernel.last_results = None


# revision 3
# speedup vs baseline: 3.1568x; 3.1568x over previous
"""Trainium2 Bass kernel for HandmadeConv2d.

Conv2d NCHW, valid padding, stride 1, no bias:
  x: (32, 128, 64, 64) f32, weights: (256, 128, 3, 3) f32 -> out: (32, 256, 62, 62) f32

Sharding: data-parallel over batch, 4 images per core across 8 NeuronCores;
weights replicated.

Per core the conv is computed as 9 accumulating matmuls per output tile:
  out[oc, (oh,ow)] += W[kh,kw][ic, oc].T @ x[ic, (oh+kh, ow+kw)]
with ic=128 as the PE contraction dim, oc split into 2 chunks of 128
(PSUM partition dim), and spatial tiled as 8 output rows x 62 cols = 496
moving-operand elements (<=512 fp32 limit, fits one PSUM bank).

Weight [ic, oc] tiles are built on-chip with PE transposes of the
contiguously-DMA'd [oc, ic] layout.

Matmul dtype modes (BASS_CONV_MODE env var):
  fp32      - native fp32 matmul (4 cycles/row), ~1e-7 rel err
  fp32r     - single-pass rounded fp32 (1 cycle/row), ~1.5e-4 rel err
  fp32rsplit- hi/lo fp32r decomposition, 3 matmuls (~3 cycles/row), ~1e-7 rel err
  bf16split - hi/lo bf16 decomposition, 3 matmuls (~3 cycles/row), ~5e-6 rel err
"""

import os
import warnings

warnings.filterwarnings("ignore")

import numpy as np

N_CORES = 8
NIMG = 4  # images per core
IC = 128
OC = 256
H = W = 64
OH = OW = 62
P = 128

MODE = os.environ.get("BASS_CONV_MODE", "fp32rsplit")

_NC_CACHE = {}


def _row_groups():
    groups = []
    r = 0
    while r < OH:
        nr = min(8, OH - r)
        groups.append((r, nr))
        r += nr
    return groups


def build_nc(mode):
    import concourse.bacc as bacc
    import concourse.mybir as mybir
    import concourse.tile as tile
    from concourse.masks import make_identity

    f32 = mybir.dt.float32
    f32r = mybir.dt.float32r
    bf16 = mybir.dt.bfloat16

    nc = bacc.Bacc("TRN2", target_bir_lowering=False, debug=False)
    x = nc.dram_tensor("x", [NIMG, IC, H, W], f32, kind="ExternalInput")
    w = nc.dram_tensor("w", [OC, IC, 3, 3], f32, kind="ExternalInput")
    out = nc.dram_tensor("out", [NIMG, OC, OH, OW], f32, kind="ExternalOutput")

    split = mode in ("fp32rsplit", "bf16split")
    if mode == "fp32":
        mm_dt = f32
    elif mode == "fp32r":
        mm_dt = f32r
    elif mode == "fp32rsplit":
        mm_dt = f32r
    elif mode == "bf16split":
        mm_dt = bf16
    else:
        raise ValueError(mode)

    groups = _row_groups()

    with tile.TileContext(nc) as tc:
        with (
            tc.tile_pool(name="wstage", bufs=1) as wstage,
            tc.tile_pool(name="wtiles", bufs=1) as wtiles,
            tc.tile_pool(name="xstage", bufs=4) as xstage,
            tc.tile_pool(name="xconv", bufs=6) as xconv,
            tc.tile_pool(name="osb", bufs=8) as osb,
            tc.tile_pool(name="consts", bufs=1) as consts,
            tc.tile_pool(name="pst", bufs=2, space="PSUM") as pst,
            tc.tile_pool(name="psmm", bufs=6, space="PSUM") as psmm,
        ):
            # ---- weights: DMA contiguous [oc, ic*9], PE-transpose to [ic, oc] tiles
            ident = consts.tile([P, P], f32)
            make_identity(nc, ident[:])

            wsb = wstage.tile([P, 2, IC * 9], f32)
            for c in range(2):
                nc.sync.dma_start(
                    wsb[:, c, :],
                    w[:][c * P : (c + 1) * P].rearrange("o i h k -> o (i h k)"),
                )
            wsb_r = wsb[:].rearrange("o c (i k) -> o c i k", k=9)

            wt_hi = wtiles.tile([P, 2, 9, P], mm_dt, tag="wt_hi")
            if split:
                wt_lo = wtiles.tile([P, 2, 9, P], mm_dt, tag="wt_lo")
                wtmp = wtiles.tile([P, P], f32, tag="wtmp", bufs=2)

            for c in range(2):
                for k in range(9):
                    ptile = pst.tile([P, P], f32)
                    nc.tensor.transpose(ptile[:], wsb_r[:, c, :, k], ident[:])
                    hi_slice = wt_hi[:, c, k, :]
                    nc.vector.tensor_copy(hi_slice, ptile[:])
                    if split:
                        if mode == "fp32rsplit":
                            hi32 = hi_slice.bitcast(f32)
                        else:
                            hi32 = wtiles.tile([P, P], f32, tag="hi32", bufs=2)[:]
                            nc.vector.tensor_copy(hi32, hi_slice)
                        lo32 = wtmp[:]
                        nc.vector.tensor_sub(lo32, ptile[:], hi32)
                        nc.vector.tensor_copy(wt_lo[:, c, k, :], lo32)
                        wtmp = wtiles.tile([P, P], f32, tag="wtmp", bufs=2)

            # ---- main loop over images
            # x is loaded and converted in 4 row-bands (2-row halo) so the
            # first matmuls start after ~1/4 of an image is resident and the
            # dtype-conversion casts pipeline in small chunks.
            BANDS = [(0, 18), (16, 18), (32, 18), (48, 16)]  # (row0, nrows)

            def band_of(r0):
                return min(3, r0 // 16)

            for n in range(NIMG):
                xb_terms = []  # per band: list of term tiles
                for b0, bn in BANDS:
                    stage = xstage.tile([P, 18, W], f32, tag="xs")
                    nc.sync.dma_start(stage[:, :bn, :], x[:][n, :, b0 : b0 + bn, :])

                    if mode == "fp32":
                        bt_terms = [stage]
                    elif mode == "fp32r":
                        xhi = xconv.tile([P, 18, W], f32r, tag="xhi")
                        nc.vector.tensor_copy(xhi[:, :bn, :], stage[:, :bn, :])
                        bt_terms = [xhi]
                    else:
                        xhi = xconv.tile([P, 18, W], mm_dt, tag="xhi")
                        nc.vector.tensor_copy(xhi[:, :bn, :], stage[:, :bn, :])
                        if mode == "fp32rsplit":
                            hi32v = xhi[:, :bn, :].bitcast(f32)
                        else:
                            hi32 = xconv.tile([P, 18, W], f32, tag="xhi32")
                            nc.vector.tensor_copy(hi32[:, :bn, :], xhi[:, :bn, :])
                            hi32v = hi32[:, :bn, :]
                        lo32 = xconv.tile([P, 18, W], f32, tag="xlo32")
                        nc.vector.tensor_sub(lo32[:, :bn, :], stage[:, :bn, :], hi32v)
                        xlo = xconv.tile([P, 18, W], mm_dt, tag="xlo")
                        nc.vector.tensor_copy(xlo[:, :bn, :], lo32[:, :bn, :])
                        bt_terms = [xhi, xlo]
                    xb_terms.append(bt_terms)

                nterms = 3 if split else 1

                for c in range(2):
                    for r0, nr in groups:
                        b = band_of(r0)
                        b0 = BANDS[b][0]
                        xts = xb_terms[b]
                        if split:
                            terms = [(wt_hi, xts[0]), (wt_hi, xts[1]), (wt_lo, xts[0])]
                        else:
                            terms = [(wt_hi, xts[0])]
                        ps_t = psmm.tile([P, 8 * OW], f32, tag="mm")
                        nmm = nterms * 9
                        i = 0
                        for wt, xt in terms:
                            for kh in range(3):
                                for kw in range(3):
                                    rr = r0 - b0 + kh
                                    nc.tensor.matmul(
                                        ps_t[:, : nr * OW],
                                        wt[:, c, kh * 3 + kw, :],
                                        xt[:, rr : rr + nr, kw : kw + OW],
                                        start=(i == 0),
                                        stop=(i == nmm - 1),
                                    )
                                    i += 1
                        ob = osb.tile([P, 8 * OW], f32, tag="ob")
                        nc.any.tensor_copy(ob[:, : nr * OW], ps_t[:, : nr * OW])
                        nc.sync.dma_start(
                            out[:][n, c * P : (c + 1) * P, r0 : r0 + nr, :],
                            ob[:, : nr * OW].rearrange("p (r q) -> p r q", q=OW),
                        )

    nc.compile()
    return nc


def get_nc(mode=None):
    mode = mode or MODE
    if mode not in _NC_CACHE:
        _NC_CACHE[mode] = build_nc(mode)
    return _NC_CACHE[mode]


def kernel(x, weights, _trace=False, _mode=None):
    from concourse.bass_utils import run_bass_kernel_spmd

    nc = get_nc(_mode)
    x = np.ascontiguousarray(np.asarray(x), dtype=np.float32)
    weights = np.ascontiguousarray(np.asarray(weights), dtype=np.float32)
    in_maps = [
        {"x": x[i * NIMG : (i + 1) * NIMG], "w": weights} for i in range(N_CORES)
    ]
    res = run_bass_kernel_spmd(
        nc, in_maps, core_ids=list(range(N_CORES)), trace=_trace
    )
    out = np.concatenate([r["out"] for r in res.results], axis=0)
    if _trace:
        kernel.last_results = res
    return out


kernel.last_results = None


# revision 4
# speedup vs baseline: 3.1997x; 1.0136x over previous
"""Trainium2 Bass kernel for HandmadeConv2d.

Conv2d NCHW, valid padding, stride 1, no bias:
  x: (32, 128, 64, 64) f32, weights: (256, 128, 3, 3) f32 -> out: (32, 256, 62, 62) f32

Sharding: data-parallel over batch, 4 images per core across 8 NeuronCores;
weights replicated.

Per core the conv is computed as 9 accumulating matmuls per output tile:
  out[oc, (oh,ow)] += W[kh,kw][ic, oc].T @ x[ic, (oh+kh, ow+kw)]
with ic=128 as the PE contraction dim, oc split into 2 chunks of 128
(PSUM partition dim), and spatial tiled as 8 output rows x 62 cols = 496
moving-operand elements (<=512 fp32 limit, fits one PSUM bank).

All data preparation happens on the host: weights are pre-transposed to
[ic, kh*kw, oc] (so they DMA straight into the stationary-operand layout)
and, for the fp32r modes, operands are pre-rounded to the PE's fp32r
format (round-to-nearest-even keeping 11 mantissa bits) so the device
performs zero weight transposes and zero dtype casts.

Matmul dtype modes (BASS_CONV_MODE env var):
  fp32      - native fp32 matmul (4 cycles/row), bitwise-matches the jax
              reference on TRN2
  fp32r     - single-pass rounded fp32 (1 cycle/row), ~1.4e-4 rel err
  fp32rsplit- hi/lo fp32r decomposition, 3 matmuls, ~2e-7 rel err
  bf16split - hi/lo bf16 decomposition, 3 matmuls, ~5e-6 rel err
"""

import os
import warnings

warnings.filterwarnings("ignore")

import numpy as np

N_CORES = 8
NIMG = 4  # images per core
IC = 128
OC = 256
H = W = 64
OH = OW = 62
P = 128

MODE = os.environ.get("BASS_CONV_MODE", "fp32r")

_NC_CACHE = {}

# x row-bands (2-row halo) so first matmuls start after ~1/4 image is resident
BANDS = [(0, 18), (16, 18), (32, 18), (48, 16)]  # (row0, nrows)


def _row_groups():
    groups = []
    r = 0
    while r < OH:
        nr = min(8, OH - r)
        groups.append((r, nr))
        r += nr
    return groups


def round_fp32r(a):
    """Round fp32 to the PE's fp32r format: RNE keeping 11 mantissa bits.
    Matches the hardware's rounding (validated bit-level on TRN2)."""
    u = np.ascontiguousarray(a, dtype=np.float32).view(np.uint32)
    low = u & np.uint32(0xFFF)
    base = u & np.uint32(0xFFFFF000)
    lsb = (u >> np.uint32(12)) & np.uint32(1)
    up = (low > 0x800) | ((low == 0x800) & (lsb == 1))
    r = base + (up.astype(np.uint32) << np.uint32(12))
    return r.view(np.float32).reshape(a.shape)


def build_nc(mode):
    import concourse.bacc as bacc
    import concourse.mybir as mybir
    import concourse.tile as tile

    f32 = mybir.dt.float32
    if mode == "fp32":
        ddt = f32
    elif mode in ("fp32r", "fp32rsplit"):
        ddt = mybir.dt.float32r
    elif mode == "bf16split":
        ddt = mybir.dt.bfloat16
    else:
        raise ValueError(mode)
    split = mode in ("fp32rsplit", "bf16split")

    nc = bacc.Bacc("TRN2", target_bir_lowering=False, debug=False)
    xh = nc.dram_tensor("xh", [NIMG, IC, H, W], ddt, kind="ExternalInput")
    wh = nc.dram_tensor("wh", [IC, 9, OC], ddt, kind="ExternalInput")
    if split:
        xl = nc.dram_tensor("xl", [NIMG, IC, H, W], ddt, kind="ExternalInput")
        wl = nc.dram_tensor("wl", [IC, 9, OC], ddt, kind="ExternalInput")
    out = nc.dram_tensor("out", [NIMG, OC, OH, OW], f32, kind="ExternalOutput")

    groups = _row_groups()

    with tile.TileContext(nc) as tc:
        with (
            tc.tile_pool(name="wtiles", bufs=1) as wtiles,
            tc.tile_pool(name="xconv", bufs=6) as xconv,
            tc.tile_pool(name="osb", bufs=8) as osb,
            tc.tile_pool(name="psmm", bufs=8, space="PSUM") as psmm,
        ):
            wt_hi = wtiles.tile([P, 9, OC], ddt, tag="wt_hi")
            nc.sync.dma_start(wt_hi[:], wh[:])
            if split:
                wt_lo = wtiles.tile([P, 9, OC], ddt, tag="wt_lo")
                nc.sync.dma_start(wt_lo[:], wl[:])

            for n in range(NIMG):
                xb_terms = []  # per band: [hi_tile, (lo_tile)]
                for b0, bn in BANDS:
                    bhi = xconv.tile([P, 18, W], ddt, tag="xbh")
                    nc.sync.dma_start(bhi[:, :bn, :], xh[:][n, :, b0 : b0 + bn, :])
                    terms_b = [bhi]
                    if split:
                        blo = xconv.tile([P, 18, W], ddt, tag="xbl")
                        nc.sync.dma_start(blo[:, :bn, :], xl[:][n, :, b0 : b0 + bn, :])
                        terms_b.append(blo)
                    xb_terms.append(terms_b)

                for c in range(2):
                    for r0, nr in groups:
                        b = min(3, r0 // 16)
                        b0 = BANDS[b][0]
                        xts = xb_terms[b]
                        if split:
                            terms = [(wt_hi, xts[0]), (wt_hi, xts[1]), (wt_lo, xts[0])]
                        else:
                            terms = [(wt_hi, xts[0])]
                        ps_t = psmm.tile([P, 8 * OW], mybir.dt.float32, tag="mm")
                        nmm = len(terms) * 9
                        i = 0
                        for wt, xt in terms:
                            for k in range(9):
                                kh, kw = divmod(k, 3)
                                rr = r0 - b0 + kh
                                nc.tensor.matmul(
                                    ps_t[:, : nr * OW],
                                    wt[:, k, c * P : (c + 1) * P],
                                    xt[:, rr : rr + nr, kw : kw + OW],
                                    start=(i == 0),
                                    stop=(i == nmm - 1),
                                )
                                i += 1
                        ob = osb.tile([P, 8 * OW], mybir.dt.float32, tag="ob")
                        nc.any.tensor_copy(ob[:, : nr * OW], ps_t[:, : nr * OW])
                        nc.sync.dma_start(
                            out[:][n, c * P : (c + 1) * P, r0 : r0 + nr, :],
                            ob[:, : nr * OW].rearrange("p (r q) -> p r q", q=OW),
                        )

    nc.compile()
    return nc


def get_nc(mode=None):
    mode = mode or MODE
    if mode not in _NC_CACHE:
        _NC_CACHE[mode] = build_nc(mode)
    return _NC_CACHE[mode]


def _host_prep(x, weights, mode):
    """Host-side data prep: weight transpose to [ic, kh*kw, oc] plus
    per-mode rounding / hi-lo decomposition."""
    x = np.ascontiguousarray(np.asarray(x), dtype=np.float32)
    w = np.ascontiguousarray(np.asarray(weights), dtype=np.float32)
    wt = np.ascontiguousarray(w.transpose(1, 2, 3, 0)).reshape(IC, 9, OC)

    if mode == "fp32":
        return {"xh": x, "wh": wt}
    if mode == "fp32r":
        return {"xh": round_fp32r(x), "wh": round_fp32r(wt)}
    if mode == "fp32rsplit":
        xhi = round_fp32r(x)
        whi = round_fp32r(wt)
        return {
            "xh": xhi,
            "xl": round_fp32r(x - xhi),
            "wh": whi,
            "wl": round_fp32r(wt - whi),
        }
    if mode == "bf16split":
        import ml_dtypes

        bf = ml_dtypes.bfloat16
        xhi = x.astype(bf)
        whi = wt.astype(bf)
        xlo = (x - xhi.astype(np.float32)).astype(bf)
        wlo = (wt - whi.astype(np.float32)).astype(bf)
        return {"xh": xhi, "xl": xlo, "wh": whi, "wl": wlo}
    raise ValueError(mode)


def kernel(x, weights, _trace=False, _mode=None):
    from concourse.bass_utils import run_bass_kernel_spmd

    mode = _mode or MODE
    nc = get_nc(mode)
    tensors = _host_prep(x, weights, mode)
    in_maps = []
    for i in range(N_CORES):
        m = {}
        for k, v in tensors.items():
            m[k] = v[i * NIMG : (i + 1) * NIMG] if k.startswith("x") else v
        in_maps.append(m)
    res = run_bass_kernel_spmd(
        nc, in_maps, core_ids=list(range(N_CORES)), trace=_trace
    )
    out = np.concatenate([r["out"] for r in res.results], axis=0)
    if _trace:
        kernel.last_results = res
    return out


kernel.last_results = None


# revision 6
# speedup vs baseline: 3.2137x; 1.0044x over previous
"""Trainium2 Bass kernel for HandmadeConv2d.

Conv2d NCHW, valid padding, stride 1, no bias:
  x: (32, 128, 64, 64) f32, weights: (256, 128, 3, 3) f32 -> out: (32, 256, 62, 62) f32

Sharding: data-parallel over batch, 4 images per core across 8 NeuronCores;
weights replicated.

Per core the conv is computed as 9 accumulating matmuls per output tile:
  out[oc, (oh,ow)] += W[kh,kw][ic, oc].T @ x[ic, (oh+kh, ow+kw)]
with ic=128 as the PE contraction dim, oc split into 2 chunks of 128
(PSUM partition dim), and spatial tiled as 8 output rows x 62 cols = 496
moving-operand elements (<=512 fp32 limit, fits one PSUM bank).

All data preparation happens on the host: weights are pre-transposed to
[ic, kh*kw, oc] (so they DMA straight into the stationary-operand layout)
and, for the fp32r modes, operands are pre-rounded to the PE's fp32r
format (round-to-nearest-even keeping 11 mantissa bits) so the device
performs zero weight transposes and zero dtype casts.

Matmul dtype modes (BASS_CONV_MODE env var):
  fp32      - native fp32 matmul (4 cycles/row), bitwise-matches the jax
              reference on TRN2
  fp32r     - single-pass rounded fp32 (1 cycle/row), ~1.4e-4 rel err
  fp32rsplit- hi/lo fp32r decomposition, 3 matmuls, ~2e-7 rel err
  bf16split - hi/lo bf16 decomposition, 3 matmuls, ~5e-6 rel err
"""

import os
import warnings

warnings.filterwarnings("ignore")

import numpy as np

N_CORES = 8
NIMG = 4  # images per core
IC = 128
OC = 256
H = W = 64
OH = OW = 62
P = 128

MODE = os.environ.get("BASS_CONV_MODE", "fp32r")

_NC_CACHE = {}

# x row-bands (2-row halo) so first matmuls start after ~1/4 image is resident
BANDS = [(0, 18), (16, 18), (32, 18), (48, 16)]  # (row0, nrows)


def _row_groups():
    groups = []
    r = 0
    while r < OH:
        nr = min(8, OH - r)
        groups.append((r, nr))
        r += nr
    return groups


def round_fp32r(a):
    """Round fp32 to the PE's fp32r format: RNE keeping 11 mantissa bits.
    Matches the hardware's rounding (validated bit-level on TRN2)."""
    u = np.ascontiguousarray(a, dtype=np.float32).view(np.uint32)
    low = u & np.uint32(0xFFF)
    base = u & np.uint32(0xFFFFF000)
    lsb = (u >> np.uint32(12)) & np.uint32(1)
    up = (low > 0x800) | ((low == 0x800) & (lsb == 1))
    r = base + (up.astype(np.uint32) << np.uint32(12))
    return r.view(np.float32).reshape(a.shape)


def build_nc(mode):
    import concourse.bacc as bacc
    import concourse.mybir as mybir
    import concourse.tile as tile

    f32 = mybir.dt.float32
    if mode == "fp32":
        ddt = f32
    elif mode in ("fp32r", "fp32rsplit"):
        ddt = mybir.dt.float32r
    elif mode == "bf16split":
        ddt = mybir.dt.bfloat16
    else:
        raise ValueError(mode)
    split = mode in ("fp32rsplit", "bf16split")

    nc = bacc.Bacc("TRN2", target_bir_lowering=False, debug=False)
    xh = nc.dram_tensor("xh", [NIMG, IC, H, W], ddt, kind="ExternalInput")
    wh = nc.dram_tensor("wh", [IC, 9, OC], ddt, kind="ExternalInput")
    if split:
        xl = nc.dram_tensor("xl", [NIMG, IC, H, W], ddt, kind="ExternalInput")
        wl = nc.dram_tensor("wl", [IC, 9, OC], ddt, kind="ExternalInput")
    out = nc.dram_tensor("out", [NIMG, OC, OH, OW], f32, kind="ExternalOutput")

    groups = _row_groups()

    with tile.TileContext(nc) as tc:
        with (
            tc.tile_pool(name="wtiles", bufs=1) as wtiles,
            tc.tile_pool(name="xconv", bufs=8) as xconv,
            tc.tile_pool(name="osb", bufs=8) as osb,
            tc.tile_pool(name="psmm", bufs=8, space="PSUM") as psmm,
        ):
            # startup-ordered DMAs: first x band, then weights in 3 chunks
            # (first matmul only needs band 0 + the k=0..2 weight slice), so
            # the PE starts ~4us earlier than with one monolithic weight DMA.
            def load_bands(n, first=False):
                terms = []
                for bi, (b0, bn) in enumerate(BANDS):
                    bhi = xconv.tile([P, 18, W], ddt, tag="xbh")
                    nc.sync.dma_start(bhi[:, :bn, :], xh[:][n, :, b0 : b0 + bn, :])
                    terms_b = [bhi]
                    if split:
                        blo = xconv.tile([P, 18, W], ddt, tag="xbl")
                        nc.sync.dma_start(blo[:, :bn, :], xl[:][n, :, b0 : b0 + bn, :])
                        terms_b.append(blo)
                    terms.append(terms_b)
                    if first and bi == 0:
                        _load_weights()
                return terms

            wt_hi = wtiles.tile([P, 9, OC], ddt, tag="wt_hi")
            if split:
                wt_lo = wtiles.tile([P, 9, OC], ddt, tag="wt_lo")

            def _load_weights():
                for k0 in range(0, 9, 3):
                    nc.sync.dma_start(wt_hi[:, k0 : k0 + 3, :], wh[:][:, k0 : k0 + 3, :])
                    if split:
                        nc.sync.dma_start(
                            wt_lo[:, k0 : k0 + 3, :], wl[:][:, k0 : k0 + 3, :]
                        )

            for n in range(NIMG):
                xb_terms = load_bands(n, first=(n == 0))

                for c in range(2):
                    for r0, nr in groups:
                        b = min(3, r0 // 16)
                        b0 = BANDS[b][0]
                        xts = xb_terms[b]
                        if split:
                            terms = [(wt_hi, xts[0]), (wt_hi, xts[1]), (wt_lo, xts[0])]
                        else:
                            terms = [(wt_hi, xts[0])]
                        ps_t = psmm.tile([P, 8 * OW], mybir.dt.float32, tag="mm")
                        nmm = len(terms) * 9
                        i = 0
                        for wt, xt in terms:
                            for k in range(9):
                                kh, kw = divmod(k, 3)
                                rr = r0 - b0 + kh
                                nc.tensor.matmul(
                                    ps_t[:, : nr * OW],
                                    wt[:, k, c * P : (c + 1) * P],
                                    xt[:, rr : rr + nr, kw : kw + OW],
                                    start=(i == 0),
                                    stop=(i == nmm - 1),
                                )
                                i += 1
                        ob = osb.tile([P, 8 * OW], mybir.dt.float32, tag="ob")
                        nc.any.tensor_copy(ob[:, : nr * OW], ps_t[:, : nr * OW])
                        nc.sync.dma_start(
                            out[:][n, c * P : (c + 1) * P, r0 : r0 + nr, :],
                            ob[:, : nr * OW].rearrange("p (r q) -> p r q", q=OW),
                        )

    nc.compile()
    return nc


def get_nc(mode=None):
    mode = mode or MODE
    if mode not in _NC_CACHE:
        _NC_CACHE[mode] = build_nc(mode)
    return _NC_CACHE[mode]


def _host_prep(x, weights, mode):
    """Host-side data prep: weight transpose to [ic, kh*kw, oc] plus
    per-mode rounding / hi-lo decomposition."""
    x = np.ascontiguousarray(np.asarray(x), dtype=np.float32)
    w = np.ascontiguousarray(np.asarray(weights), dtype=np.float32)
    wt = np.ascontiguousarray(w.transpose(1, 2, 3, 0)).reshape(IC, 9, OC)

    if mode == "fp32":
        return {"xh": x, "wh": wt}
    if mode == "fp32r":
        return {"xh": round_fp32r(x), "wh": round_fp32r(wt)}
    if mode == "fp32rsplit":
        xhi = round_fp32r(x)
        whi = round_fp32r(wt)
        return {
            "xh": xhi,
            "xl": round_fp32r(x - xhi),
            "wh": whi,
            "wl": round_fp32r(wt - whi),
        }
    if mode == "bf16split":
        import ml_dtypes

        bf = ml_dtypes.bfloat16
        xhi = x.astype(bf)
        whi = wt.astype(bf)
        xlo = (x - xhi.astype(np.float32)).astype(bf)
        wlo = (wt - whi.astype(np.float32)).astype(bf)
        return {"xh": xhi, "xl": xlo, "wh": whi, "wl": wlo}
    raise ValueError(mode)


def kernel(x, weights, _trace=False, _mode=None):
    from concourse.bass_utils import run_bass_kernel_spmd

    mode = _mode or MODE
    nc = get_nc(mode)
    tensors = _host_prep(x, weights, mode)
    in_maps = []
    for i in range(N_CORES):
        m = {}
        for k, v in tensors.items():
            m[k] = v[i * NIMG : (i + 1) * NIMG] if k.startswith("x") else v
        in_maps.append(m)
    res = run_bass_kernel_spmd(
        nc, in_maps, core_ids=list(range(N_CORES)), trace=_trace
    )
    out = np.concatenate([r["out"] for r in res.results], axis=0)
    if _trace:
        kernel.last_results = res
    return out


kernel.last_results = None


# revision 8
# speedup vs baseline: 3.2143x; 1.0002x over previous
"""Trainium2 Bass kernel for HandmadeConv2d.

Conv2d NCHW, valid padding, stride 1, no bias:
  x: (32, 128, 64, 64) f32, weights: (256, 128, 3, 3) f32 -> out: (32, 256, 62, 62) f32

Sharding: data-parallel over batch, 4 images per core across 8 NeuronCores;
weights replicated.

Per core the conv is computed as 9 accumulating matmuls per output tile:
  out[oc, (oh,ow)] += W[kh,kw][ic, oc].T @ x[ic, (oh+kh, ow+kw)]
with ic=128 as the PE contraction dim, oc split into 2 chunks of 128
(PSUM partition dim), and spatial tiled as 8 output rows x 62 cols = 496
moving-operand elements (<=512 fp32 limit, fits one PSUM bank).

All data preparation happens on the host: weights are pre-transposed to
[ic, kh*kw, oc] (so they DMA straight into the stationary-operand layout)
and, for the fp32r modes, operands are pre-rounded to the PE's fp32r
format (round-to-nearest-even keeping 11 mantissa bits) so the device
performs zero weight transposes and zero dtype casts.

Matmul dtype modes (BASS_CONV_MODE env var):
  fp32      - native fp32 matmul (4 cycles/row), bitwise-matches the jax
              reference on TRN2
  fp32r     - single-pass rounded fp32 (1 cycle/row), ~1.4e-4 rel err
  fp32rsplit- hi/lo fp32r decomposition, 3 matmuls, ~2e-7 rel err
  bf16split - hi/lo bf16 decomposition, 3 matmuls, ~5e-6 rel err
"""

import os
import warnings

warnings.filterwarnings("ignore")

import numpy as np

N_CORES = 8
NIMG = 4  # images per core
IC = 128
OC = 256
H = W = 64
OH = OW = 62
P = 128

MODE = os.environ.get("BASS_CONV_MODE", "fp32r")

_NC_CACHE = {}

# x row-bands (2-row halo) so first matmuls start after ~1/4 image is resident
BANDS = [(0, 18), (16, 18), (32, 18), (48, 16)]  # (row0, nrows)


def _row_groups():
    groups = []
    r = 0
    while r < OH:
        nr = min(8, OH - r)
        groups.append((r, nr))
        r += nr
    return groups


def round_fp32r(a):
    """Round fp32 to the PE's fp32r format: RNE keeping 11 mantissa bits.
    Matches the hardware's rounding (validated bit-level on TRN2)."""
    u = np.ascontiguousarray(a, dtype=np.float32).view(np.uint32)
    low = u & np.uint32(0xFFF)
    base = u & np.uint32(0xFFFFF000)
    lsb = (u >> np.uint32(12)) & np.uint32(1)
    up = (low > 0x800) | ((low == 0x800) & (lsb == 1))
    r = base + (up.astype(np.uint32) << np.uint32(12))
    return r.view(np.float32).reshape(a.shape)


def build_nc(mode):
    import concourse.bacc as bacc
    import concourse.mybir as mybir
    import concourse.tile as tile

    f32 = mybir.dt.float32
    if mode == "fp32":
        ddt = f32
    elif mode in ("fp32r", "fp32rsplit"):
        ddt = mybir.dt.float32r
    elif mode == "bf16split":
        ddt = mybir.dt.bfloat16
    else:
        raise ValueError(mode)
    split = mode in ("fp32rsplit", "bf16split")

    nc = bacc.Bacc("TRN2", target_bir_lowering=False, debug=False)
    xh = nc.dram_tensor("xh", [NIMG, IC, H, W], ddt, kind="ExternalInput")
    wh = nc.dram_tensor("wh", [IC, 9, OC], ddt, kind="ExternalInput")
    if split:
        xl = nc.dram_tensor("xl", [NIMG, IC, H, W], ddt, kind="ExternalInput")
        wl = nc.dram_tensor("wl", [IC, 9, OC], ddt, kind="ExternalInput")
    out = nc.dram_tensor("out", [NIMG, OC, OH, OW], f32, kind="ExternalOutput")

    groups = _row_groups()

    with tile.TileContext(nc) as tc:
        with (
            tc.tile_pool(name="wtiles", bufs=1) as wtiles,
            tc.tile_pool(name="xconv", bufs=8) as xconv,
            tc.tile_pool(name="osb", bufs=8) as osb,
            tc.tile_pool(name="psmm", bufs=8, space="PSUM") as psmm,
        ):
            # startup-ordered DMAs: first x band, then weights in 3 chunks
            # (first matmul only needs band 0 + the k=0..2 weight slice), so
            # the PE starts ~4us earlier than with one monolithic weight DMA.
            def load_bands(n, engine=None):
                eng = engine or nc.sync
                terms = []
                for b0, bn in BANDS:
                    bhi = xconv.tile([P, 18, W], ddt, tag="xbh")
                    eng.dma_start(bhi[:, :bn, :], xh[:][n, :, b0 : b0 + bn, :])
                    terms_b = [bhi]
                    if split:
                        blo = xconv.tile([P, 18, W], ddt, tag="xbl")
                        eng.dma_start(blo[:, :bn, :], xl[:][n, :, b0 : b0 + bn, :])
                        terms_b.append(blo)
                    terms.append(terms_b)
                return terms

            wt_hi = wtiles.tile([P, 9, OC], ddt, tag="wt_hi")
            if split:
                wt_lo = wtiles.tile([P, 9, OC], ddt, tag="wt_lo")

            # weights issued on Sync first (they are the startup critical
            # path); image-0 bands issued concurrently from GpSimd's queue.
            for k0 in range(0, 9, 3):
                nc.sync.dma_start(wt_hi[:, k0 : k0 + 3, :], wh[:][:, k0 : k0 + 3, :])
                if split:
                    nc.sync.dma_start(
                        wt_lo[:, k0 : k0 + 3, :], wl[:][:, k0 : k0 + 3, :]
                    )

            # PE pre-warm: ~4us of dummy matmuls on a zeroed tile during the
            # initial DMA wait, so HAM un-throttles the PE clock (1.2->2.4
            # GHz) before the first real matmul issues.
            warm = wtiles.tile([P, 256], mybir.dt.bfloat16, tag="warm")
            nc.gpsimd.memset(warm[:], 0.0)
            for _ in range(18):
                wps = psmm.tile([P, 8 * OW], mybir.dt.float32, tag="mm")
                nc.tensor.matmul(
                    wps[:, :256], warm[:, :P], warm[:, :256], start=True, stop=True
                )

            for n in range(NIMG):
                xb_terms = load_bands(n, engine=nc.gpsimd if n == 0 else None)

                for c in range(2):
                    for r0, nr in groups:
                        b = min(3, r0 // 16)
                        b0 = BANDS[b][0]
                        xts = xb_terms[b]
                        if split:
                            terms = [(wt_hi, xts[0]), (wt_hi, xts[1]), (wt_lo, xts[0])]
                        else:
                            terms = [(wt_hi, xts[0])]
                        ps_t = psmm.tile([P, 8 * OW], mybir.dt.float32, tag="mm")
                        nmm = len(terms) * 9
                        i = 0
                        for wt, xt in terms:
                            for k in range(9):
                                kh, kw = divmod(k, 3)
                                rr = r0 - b0 + kh
                                nc.tensor.matmul(
                                    ps_t[:, : nr * OW],
                                    wt[:, k, c * P : (c + 1) * P],
                                    xt[:, rr : rr + nr, kw : kw + OW],
                                    start=(i == 0),
                                    stop=(i == nmm - 1),
                                )
                                i += 1
                        ob = osb.tile([P, 8 * OW], mybir.dt.float32, tag="ob")
                        nc.any.tensor_copy(ob[:, : nr * OW], ps_t[:, : nr * OW])
                        nc.sync.dma_start(
                            out[:][n, c * P : (c + 1) * P, r0 : r0 + nr, :],
                            ob[:, : nr * OW].rearrange("p (r q) -> p r q", q=OW),
                        )

    nc.compile()
    return nc


def get_nc(mode=None):
    mode = mode or MODE
    if mode not in _NC_CACHE:
        _NC_CACHE[mode] = build_nc(mode)
    return _NC_CACHE[mode]


def _host_prep(x, weights, mode):
    """Host-side data prep: weight transpose to [ic, kh*kw, oc] plus
    per-mode rounding / hi-lo decomposition."""
    x = np.ascontiguousarray(np.asarray(x), dtype=np.float32)
    w = np.ascontiguousarray(np.asarray(weights), dtype=np.float32)
    wt = np.ascontiguousarray(w.transpose(1, 2, 3, 0)).reshape(IC, 9, OC)

    if mode == "fp32":
        return {"xh": x, "wh": wt}
    if mode == "fp32r":
        return {"xh": round_fp32r(x), "wh": round_fp32r(wt)}
    if mode == "fp32rsplit":
        xhi = round_fp32r(x)
        whi = round_fp32r(wt)
        return {
            "xh": xhi,
            "xl": round_fp32r(x - xhi),
            "wh": whi,
            "wl": round_fp32r(wt - whi),
        }
    if mode == "bf16split":
        import ml_dtypes

        bf = ml_dtypes.bfloat16
        xhi = x.astype(bf)
        whi = wt.astype(bf)
        xlo = (x - xhi.astype(np.float32)).astype(bf)
        wlo = (wt - whi.astype(np.float32)).astype(bf)
        return {"xh": xhi, "xl": xlo, "wh": whi, "wl": wlo}
    raise ValueError(mode)


def kernel(x, weights, _trace=False, _mode=None):
    from concourse.bass_utils import run_bass_kernel_spmd

    mode = _mode or MODE
    nc = get_nc(mode)
    tensors = _host_prep(x, weights, mode)
    in_maps = []
    for i in range(N_CORES):
        m = {}
        for k, v in tensors.items():
            m[k] = v[i * NIMG : (i + 1) * NIMG] if k.startswith("x") else v
        in_maps.append(m)
    res = run_bass_kernel_spmd(
        nc, in_maps, core_ids=list(range(N_CORES)), trace=_trace
    )
    out = np.concatenate([r["out"] for r in res.results], axis=0)
    if _trace:
        kernel.last_results = res
    return out


kernel.last_results = None
